# revision 74
# baseline (speedup 1.0000x reference)
"""Trainium2 Bass kernel for nn_CNNGRUforHorizon (CNN+BiGRU audio model).

Strategy: W-shard the logmel branch + fusion conv across 8 cores (each core
owns 64 of the 512 fused-map columns, with halo), replicate the tiny wave
branch, reduce the GRU input-gate partials with a single AllReduce, then run
the 32-step BiGRU replicated on every core.

v3 changes over the v2 baseline:
- m2 + the fc fusion conv run in fp8e4 DoubleRow (2 k-tiles per pass).
- The fc wave half is added into PSUM via a transposed-constant matmul
  (CT[w, oc] built once from xw1), so the epilogue is a plain relu+row-sum
  instead of the per-tile max(x,-c) dance.
- One AllReduce of the gx partials [128,192] f32 replaces four 16KB feat
  AllReduces (each collective pays a ~10-18us latency floor).
- GRU: r/z gate biases are pre-accumulated into PSUM by a K=64 selector
  matmul (gbT x E2), so one sigmoid ACT covers both gates; 4 scalar ACTs
  per step-pair instead of 6.
"""
import os
import sys

import numpy as np


def _ensure_concourse():
    try:
        import concourse  # noqa: F401
        return
    except ImportError:
        pass
    for p in ("/opt/trn_rl_repo", "/root/.axon_site/_ro/trn_rl_repo"):
        if os.path.isdir(p) and p not in sys.path:
            sys.path.insert(0, p)
    import concourse  # noqa: F401


NCORES = 8
LAST_RESULTS = None
_CACHE = {}
DEBUG_TAPS = False

# mel-branch flat layout: 64 row-slots of pitch 136 (134 data + 2 junk)
PIT = 136
MW = 64 * PIT          # 8704 flat data cols of m2/m3
M1N = MW + 2 * PIT + 16  # m1 sized so m2's +272 offset reads stay in range


def _resize_matrix(n_in, n_out):
    R = np.zeros((n_in, n_out), np.float64)
    for x in range(n_out):
        c = (x + 0.5) * n_in / n_out - 0.5
        i0 = int(np.floor(c))
        w1 = c - i0
        i0c = min(max(i0, 0), n_in - 1)
        i1c = min(max(i0 + 1, 0), n_in - 1)
        R[i0c, x] += 1.0 - w1
        R[i1c, x] += w1
    return R.astype(np.float32)


def _build():
    import concourse.bacc as bacc
    import concourse.mybir as mybir
    import concourse.tile as tile

    f32 = mybir.dt.float32
    bf16 = mybir.dt.bfloat16
    f8 = mybir.dt.float8e4
    AF = mybir.ActivationFunctionType
    ALU = mybir.AluOpType
    AX = mybir.AxisListType
    DR = mybir.MatmulPerfMode.DoubleRow

    nc = bacc.Bacc("TRN2", target_bir_lowering=False, debug=False,
                   num_devices=NCORES)

    def din(name, shape, dt=bf16):
        return nc.dram_tensor(name, shape, dt, kind="ExternalInput")

    lmI_d = din("lmI", [9, MW])              # logmel im2col slice, per core
    wP_d = din("wP", [5, 3203])              # wave, stride-5 im2col planes
    Ri_d = din("Ri", [50, 66])               # resize matrix slice, per core
    msk_d = din("msk", [128, 2], f32)        # x_mel edge-col masks, per core
    zed_d = din("zed", [128, 2], f32)        # zeros (warmup CC input)

    w1m_d = din("w1m", [9, 64])
    bn1s_d = din("bn1s", [64, 1], f32)
    bn1b_d = din("bn1b", [64, 1], f32)
    w2m_d = din("w2m", [128, 3, 2, 128], f8)     # (dx, dy-pair) DoubleRow
    bn2s_d = din("bn2s", [128, 1], f32)
    bn2b_d = din("bn2b", [128, 1], f32)
    w3m_d = din("w3m", [128, 2, 128])
    b3_d = din("b3", [128, 2], f32)
    fcm_d = din("fcm", [128, 2, 9, 4, 128], f8)  # fc mel-half weights
    fcwV_d = din("fcwV", [128, 2, 3, 3, 512])    # fc wave: dy-sum,-dy0,-dy2
    fbrow_d = din("fbrow", [1, 512])             # fb as a row
    dlt_d = din("dlt", [64, 8, 64])              # delta(w) row-broadcast rhs

    w1w_d = din("w1w", [5, 3, 64])
    wb1_d = din("wb1", [64, 1], f32)
    w2w_d = din("w2w", [64, 5, 128])
    wb2_d = din("wb2", [128, 1], f32)
    w3w_d = din("w3w", [128, 5, 2, 128])
    wb3w_d = din("wb3w", [128, 2], f32)
    w4wT_d = din("w4wT", [128, 2, 5, 256])
    wb4row_d = din("wb4row", [1, 256])
    one50_d = din("one50", [1, 50])

    I128_d = din("I128", [128, 128])
    wihT_d = din("wihT", [128, 2, 3, 4, 128])
    bhr_d = din("bhr", [1, 2, 128])          # bhh_n rows (n-gate psum const)
    brz_d = din("brz", [128, 2, 2], f32)
    bng_d = din("bng", [128, 2], f32)
    whhT_d = din("whhT", [128, 2, 3, 128])
    E2_d = din("E2", [128, 32, 2])           # (d,g,t)-selector for bias MM
    clsT_d = din("clsT", [128, 2, 5])
    clsb5_d = din("clsb5", [5, 1], f32)

    out_d = nc.dram_tensor("out", [1, 5], f32, kind="ExternalOutput")
    if DEBUG_TAPS:
        dbg_xw1_d = nc.dram_tensor("dbg_xw1", [128, 2, 66], bf16,
                                   kind="ExternalOutput")
        dbg_xmel_d = nc.dram_tensor("dbg_xmel", [128, 2, 34, 72], bf16,
                                    kind="ExternalOutput")
        dbg_featp_d = nc.dram_tensor("dbg_featp", [128, 4, 32], f32,
                                     kind="ExternalOutput")
        dbg_CT_d = nc.dram_tensor("dbg_CT", [64, 3, 512], bf16,
                                  kind="ExternalOutput")
        dbg_pgx_d = nc.dram_tensor("dbg_pgx", [128, 6, 32], f32,
                                   kind="ExternalOutput")
        dbg_hall_d = nc.dram_tensor("dbg_hall", [128, 2, 33], bf16,
                                    kind="ExternalOutput")
        dbg_gbT_d = nc.dram_tensor("dbg_gbT", [128, 128], bf16,
                                   kind="ExternalOutput")
        dbg_E2_d = nc.dram_tensor("dbg_E2", [128, 32, 2], bf16,
                                  kind="ExternalOutput")
        dbg_ps0_d = nc.dram_tensor("dbg_ps0", [128, 3, 2], f32,
                                   kind="ExternalOutput")

    with tile.TileContext(nc) as tc:
        with (
            tc.tile_pool(name="keep", bufs=1) as keep,      # long-lived
            tc.tile_pool(name="psum", bufs=1, space="PSUM") as psp,
            tc.tile_pool(name="dram", bufs=1, space="DRAM") as dram,
            tc.tile_pool(name="sc", bufs=1) as sc,          # small scratch
        ):
            # ---- long-lived tiles + weight DMAs ----
            msk = keep.tile([128, 2], f32)
            nc.scalar.dma_start(msk[:], msk_d[:])
            xmel = keep.tile([128, 2, 34, 72], f8)
            featp = keep.tile([128, 4, 32], f32)
            ftr = keep.tile([128, 4, 32], bf16)
            xw1 = keep.tile([128, 2, 66], bf16)
            CT = keep.tile([64, 3, 512], bf16)   # wave-half fc constants^T
            fcm = keep.tile([128, 2, 9, 4, 128], f8)
            fcwV = keep.tile([128, 2, 3, 3, 512], bf16)
            dlt = keep.tile([64, 8, 64], bf16)
            fbrow = keep.tile([1, 512], bf16)
            one64 = keep.tile([1, 64], bf16)
            nc.vector.memset(one64[:], 1.0)

            def psum_mm(shape):
                return psp.tile(shape, f32, tag="mm", bufs=3, name="psmm")

            def psum_m3(shape):
                return psp.tile(shape, f32, tag="m3p", bufs=2, name="psm3")

            def psum_aux(shape):
                return psp.tile(shape, f32, tag="aux", bufs=2, name="psaux")

            # warmup collective early (absorbs CC engine cold start).
            # Deliberately only ONE small warmup: ncfw cold-start varies
            # 50-150us; on bad runs a second (AR-sized) warmup serializes
            # ahead of the real feat ARs and costs far more than the few
            # microseconds it saves on good runs.
            ccz_i = dram.tile([128, 2], f32)
            ccz_o = dram.tile([128, 2], f32)
            nc.sync.dma_start(ccz_i[:], zed_d[:])
            nc.gpsimd.collective_compute(
                "AllReduce", ALU.add,
                replica_groups=[list(range(NCORES))],
                ins=[ccz_i.opt()], outs=[ccz_o.opt()])

            # mel m1 pools (allocated first so wave pool releases first)
            p_m1 = tc.alloc_tile_pool(name="mel_m1", bufs=1)
            p_rhs = tc.alloc_tile_pool(name="mel_rhs", bufs=1)
            # m1 flat [128, M1N] fp8: partitions 0-63 hold row slot r at
            # offset PIT*r (slots 0..65); partitions 64-127 hold the
            # same data shifted one row (for K=128 dy-packed m2 matmuls)
            m1 = p_m1.tile([128, M1N], f8, tag="m1")
            rhs1 = p_rhs.tile([9, MW], bf16)
            w1m = keep.tile([9, 64], bf16)
            bn1s = keep.tile([64, 1], f32)
            bn1b = keep.tile([64, 1], f32)
            w2m = keep.tile([128, 3, 2, 128], f8)
            bn2s = keep.tile([128, 1], f32)
            bn2b = keep.tile([128, 1], f32)
            w3m = keep.tile([128, 2, 128], bf16)
            b3 = keep.tile([128, 2], f32)

            nc.vector.memset(m1[0:64, 0:PIT], 0.0)
            nc.vector.memset(m1[0:64, PIT + MW:M1N], 0.0)
            nc.vector.memset(m1[64:128, MW:M1N], 0.0)

            def m1_chunk(c):
                n0 = 512 * c
                n = min(512, MW - n0)
                ps = psum_mm([64, 512])
                nc.tensor.matmul(ps[:, :n], w1m[:], rhs1[:, n0:n0 + n],
                                 start=True, stop=True)
                if c % 2 == 0:
                    nc.scalar.activation(m1[0:64, PIT + n0:PIT + n0 + n],
                                         ps[:, :n], AF.Relu,
                                         bias=bn1b[:, 0:1],
                                         scale=bn1s[:, 0:1])
                else:
                    nc.vector.tensor_scalar(
                        m1[0:64, PIT + n0:PIT + n0 + n], ps[:, :n],
                        bn1s[:, 0:1], bn1b[:, 0:1],
                        op0=ALU.mult, op1=ALU.add)
                    nc.vector.tensor_scalar_max(
                        m1[0:64, PIT + n0:PIT + n0 + n],
                        m1[0:64, PIT + n0:PIT + n0 + n], 0.0)
                # upper half (row-shifted copy) is filled by one bulk DMA
                # after the loop: m2 doesn't need it until ~40us, and the
                # per-chunk copies congested the startup-critical window

            # ============== WAVE BRANCH (replicated), interleaved with m1
            with tc.tile_pool(name="wave", bufs=1) as wv:
                Pt = wv.tile([5, 3203], bf16)
                nc.sync.dma_start(Pt[:], wP_d[:])
                w1w = wv.tile([5, 3, 64], bf16)
                nc.sync.dma_start(w1w[:], w1w_d[:])
                wb1 = wv.tile([64, 1], f32)
                nc.sync.dma_start(wb1[:], wb1_d[:])
                nc.sync.dma_start(rhs1[:], lmI_d[:])
                nc.sync.dma_start(w1m[:], w1m_d[:])
                nc.sync.dma_start(bn1s[:], bn1s_d[:])
                nc.sync.dma_start(bn1b[:], bn1b_d[:])
                w2w = wv.tile([64, 5, 128], bf16)
                nc.scalar.dma_start(w2w[:], w2w_d[:])
                wb2 = wv.tile([128, 1], f32)
                nc.scalar.dma_start(wb2[:], wb2_d[:])
                w3w = wv.tile([128, 5, 2, 128], bf16)
                nc.scalar.dma_start(w3w[:], w3w_d[:])
                wb3w = wv.tile([128, 2], f32)
                nc.scalar.dma_start(wb3w[:], wb3w_d[:])
                w4wT = wv.tile([128, 2, 5, 256], bf16)
                nc.scalar.dma_start(w4wT[:], w4wT_d[:])
                wb4row = wv.tile([1, 256], bf16)
                nc.scalar.dma_start(wb4row[:], wb4row_d[:])
                one50 = wv.tile([1, 50], bf16)
                nc.scalar.dma_start(one50[:], one50_d[:])
                Ri = wv.tile([50, 66], bf16)
                nc.scalar.dma_start(Ri[:], Ri_d[:])
                nc.sync.dma_start(w2m[:], w2m_d[:])
                nc.sync.dma_start(bn2s[:], bn2s_d[:])
                nc.sync.dma_start(bn2b[:], bn2b_d[:])
                nc.sync.dma_start(w3m[:], w3m_d[:])
                nc.sync.dma_start(b3[:], b3_d[:])
                # big fc weights on the SYNC queue, after every
                # startup-critical sync item: the scalar queue then carries
                # only small transfers, so no small-transfer dep can merge
                # behind these multi-MB loads (needed ~22us CT / ~55us fc;
                # sync's later items -- bulk m1 copy ~40us, GRU weights
                # ~150us -- have slack to absorb them)
                nc.sync.dma_start(fcwV[:], fcwV_d[:])
                nc.sync.dma_start(fcm[:], fcm_d[:])
                nc.sync.dma_start(dlt[:], dlt_d[:])
                nc.sync.dma_start(fbrow[:], fbrow_d[:])

                # conv1: 16000 -> 3200, k=11 s=5 via 3 taps of K=5
                w1o = wv.tile([64, 3200], bf16)
                for c in range(7):
                    n0 = 512 * c
                    n = min(512, 3200 - n0)
                    ps = psum_aux([64, 512])
                    for m in range(3):
                        nc.tensor.matmul(ps[:, :n], w1w[:, m, :],
                                         Pt[:, m + n0:m + n0 + n],
                                         start=(m == 0), stop=(m == 2))
                    nc.vector.tensor_scalar(w1o[:, n0:n0 + n], ps[:, :n],
                                            wb1[:, 0:1], 0.0,
                                            op0=ALU.add, op1=ALU.max)

                p1t = wv.tile([64, 804], bf16)
                t1 = wv.tile([64, 800], bf16)
                t2 = wv.tile([64, 800], bf16)
                for c in range(17):
                    m1_chunk(c)
                    if c == 0:
                        # conv1 pools issue mid-m1 so the vector queue
                        # reaches them before the PE needs p1t for conv2
                        nc.vector.memset(p1t[:, 0:2], 0.0)
                        nc.vector.memset(p1t[:, 802:804], 0.0)
                        nc.vector.tensor_tensor(t1[:], w1o[:, 0:3200:4],
                                                w1o[:, 1:3200:4],
                                                op=ALU.max)
                        nc.vector.tensor_tensor(t2[:], w1o[:, 2:3200:4],
                                                w1o[:, 3:3200:4],
                                                op=ALU.max)
                        nc.vector.tensor_tensor(p1t[:, 2:802], t1[:],
                                                t2[:], op=ALU.max)
                # upper half = lower shifted one row-slot, one bulk copy
                nc.sync.dma_start(m1[64:128, 0:MW],
                                  m1[0:64, PIT:PIT + MW])
                # conv2: k=5 pad 2, 64 -> 128 ch, 800 cols
                w2o = wv.tile([128, 800], bf16)
                for c in range(2):
                    n0 = 512 * c
                    n = min(512, 800 - n0)
                    ps = psum_aux([128, 512])
                    for tap in range(5):
                        nc.tensor.matmul(ps[:, :n], w2w[:, tap, :],
                                         p1t[:, n0 + tap:n0 + tap + n],
                                         start=(tap == 0), stop=(tap == 4))
                    nc.scalar.activation(w2o[:, n0:n0 + n], ps[:, :n],
                                         AF.Relu, bias=wb2[:, 0:1])
                p2t = wv.tile([128, 204], bf16)
                nc.vector.memset(p2t[:, 0:2], 0.0)
                nc.vector.memset(p2t[:, 202:204], 0.0)
                t3 = wv.tile([128, 200], bf16)
                t4 = wv.tile([128, 200], bf16)
                nc.vector.tensor_tensor(t3[:], w2o[:, 0:800:4],
                                        w2o[:, 1:800:4], op=ALU.max)
                nc.vector.tensor_tensor(t4[:], w2o[:, 2:800:4],
                                        w2o[:, 3:800:4], op=ALU.max)
                nc.vector.tensor_tensor(p2t[:, 2:202], t3[:], t4[:],
                                        op=ALU.max)
                # conv3: k=5 pad 2, 128 -> 256 ch, 200 cols
                w3o = wv.tile([128, 2, 200], bf16)
                for oc in range(2):
                    ps = psum_aux([128, 512])
                    for tap in range(5):
                        nc.tensor.matmul(ps[:, :200], w3w[:, tap, oc, :],
                                         p2t[:, tap:tap + 200],
                                         start=(tap == 0), stop=(tap == 4))
                    nc.scalar.activation(w3o[:, oc, :], ps[:, :200],
                                         AF.Relu, bias=wb3w[:, oc:oc + 1])
                p3t = wv.tile([128, 2, 54], bf16)
                for oc in range(2):
                    nc.vector.memset(p3t[:, oc, 0:2], 0.0)
                    nc.vector.memset(p3t[:, oc, 52:54], 0.0)
                    t5 = wv.tile([128, 50], bf16, tag="t5")
                    t6 = wv.tile([128, 50], bf16, tag="t6")
                    nc.vector.tensor_tensor(t5[:], w3o[:, oc, 0:200:4],
                                            w3o[:, oc, 1:200:4], op=ALU.max)
                    nc.vector.tensor_tensor(t6[:], w3o[:, oc, 2:200:4],
                                            w3o[:, oc, 3:200:4], op=ALU.max)
                    nc.vector.tensor_tensor(p3t[:, oc, 2:52], t5[:], t6[:],
                                            op=ALU.max)
                # conv4 (transposed out): k=5 pad 2, 256 -> 256 ch, 50 cols
                ps4 = psum_aux([50, 256])
                first = True
                for ch in range(2):
                    for tap in range(5):
                        nc.tensor.matmul(ps4[:], p3t[:, ch, tap:tap + 50],
                                         w4wT[:, ch, tap, :],
                                         start=first, stop=False)
                        first = False
                nc.tensor.matmul(ps4[:], one50[:], wb4row[:],
                                 start=False, stop=True)
                xwT = wv.tile([50, 256], bf16)
                nc.scalar.activation(xwT[:], ps4[:], AF.Relu)
                # resize 50 -> local 66 cols (per-core R slice)
                for oc in range(2):
                    psR = psum_aux([128, 66])
                    nc.tensor.matmul(psR[:], xwT[:, 128 * oc:128 * (oc + 1)],
                                     Ri[:], start=True, stop=True)
                    nc.scalar.activation(xw1[:, oc, :], psR[:], AF.Identity)

            # preload the sigmoid/tanh activation table while scalar is idle
            warm = sc.tile([1, 2], f32, tag="warm")
            nc.vector.memset(warm[:], 0.0)
            nc.scalar.activation(warm[:, 0:1], warm[:, 1:2], AF.Sigmoid)

            # ============== CT: wave-half fc constants (transposed) =======
            # CT[w, oc] = sum over (ch, dx) of xw1 x fcwV; var 0 = dy-sum
            # (+fb), var 1 = -dy0 slice (row-0 fix), var 2 = -dy2 (row-31)
            for var in range(3):
                psC = psum_aux([64, 512])
                for ch in range(2):
                    for dx in range(3):
                        last = (ch == 1 and dx == 2 and var != 0)
                        nc.tensor.matmul(psC[:], xw1[:, ch, dx:dx + 64],
                                         fcwV[:, ch, dx, var, :],
                                         start=(ch == 0 and dx == 0),
                                         stop=last)
                if var == 0:  # fb only in the dy-sum variant
                    nc.tensor.matmul(psC[:], one64[:], fbrow[:],
                                     start=False, stop=True)
                nc.scalar.activation(CT[:, var, :], psC[:], AF.Identity)

            # ============== MEL m2 / m3 (W-sharded, fp8 DoubleRow) ========
            for oc in range(2):
                nc.vector.memset(xmel[:, oc, 0, :], 0.0)
                nc.vector.memset(xmel[:, oc, 33, :], 0.0)

            p_rhs.release()
            p_m23 = tc.alloc_tile_pool(name="mel_m23", bufs=1)
            m2 = p_m23.tile([128, MW], bf16, tag="m2")
            m3 = p_m23.tile([128, 2, MW], bf16, tag="m3")
            m3v = m3.rearrange("p a (b c) -> p a b c", c=PIT)

            def pool_group(g, ocs):
                # rows 8g..8g+8 of the pooled map need m3 chunks <= 4g+4
                for oc in ocs:
                    vp = p_m23.tile([128, 8, PIT], bf16, tag="vp",
                                    bufs=2, name="vp")
                    r0 = 16 * g
                    nc.vector.tensor_tensor(
                        vp[:], m3v[:, oc, r0:r0 + 16:2, :],
                        m3v[:, oc, r0 + 1:r0 + 16:2, :], op=ALU.max)
                    if oc == 0:
                        nc.vector.tensor_tensor(
                            xmel[:, oc, 1 + 8 * g:9 + 8 * g, 0:66],
                            vp[:, :, 0:132:2], vp[:, :, 1:132:2],
                            op=ALU.max)
                    else:
                        vph = p_m23.tile([128, 8, 66], bf16, tag="vph",
                                         bufs=2, name="vph")
                        nc.vector.tensor_tensor(
                            vph[:], vp[:, :, 0:132:2], vp[:, :, 1:132:2],
                            op=ALU.max)
                        if g % 2 == 0:
                            nc.scalar.activation(
                                xmel[:, oc, 1 + 8 * g:9 + 8 * g, 0:66],
                                vph[:], AF.Relu, bias=b3[:, oc:oc + 1])
                        else:
                            nc.vector.tensor_scalar(
                                xmel[:, oc, 1 + 8 * g:9 + 8 * g, 0:66],
                                vph[:], b3[:, oc:oc + 1], 0.0,
                                op0=ALU.add, op1=ALU.max)
                    # mask halo cols per group so fc's early row groups
                    # don't wait on the whole xmel tile
                    for j, col in ((0, 0), (1, 65)):
                        nc.vector.tensor_scalar_mul(
                            xmel[:, oc, 1 + 8 * g:9 + 8 * g, col:col + 1],
                            xmel[:, oc, 1 + 8 * g:9 + 8 * g, col:col + 1],
                            msk[:, j:j + 1])

            for c in range(17):
                n0 = 512 * c
                n = min(512, MW - n0)
                ps2 = psum_mm([128, 512])
                for dx in range(3):
                    base = n0 + dx
                    rhs = m1[:, base:base + n].unsqueeze(1) \
                        .to_broadcast([128, 2, n])
                    rhs.ap[1] = [2 * PIT, 2]
                    nc.tensor.matmul(ps2[:, :n], w2m[:, dx, :, :], rhs,
                                     start=(dx == 0), stop=(dx == 2),
                                     perf_mode=DR)
                if c % 3 != 2:
                    nc.scalar.activation(m2[:, n0:n0 + n], ps2[:, :n],
                                         AF.Relu, bias=bn2b[:, 0:1],
                                         scale=bn2s[:, 0:1])
                else:
                    nc.vector.tensor_scalar(
                        m2[:, n0:n0 + n], ps2[:, :n],
                        bn2s[:, 0:1], bn2b[:, 0:1],
                        op0=ALU.mult, op1=ALU.add)
                    nc.vector.tensor_scalar_max(
                        m2[:, n0:n0 + n], m2[:, n0:n0 + n], 0.0)
                for oc in range(2):
                    ps3 = psum_m3([128, 512])
                    nc.tensor.matmul(ps3[:, :n], w3m[:, oc, :],
                                     m2[:, n0:n0 + n],
                                     start=True, stop=True)
                    if oc == 0:
                        if c % 2 == 0:
                            nc.scalar.activation(
                                m3[:, oc, n0:n0 + n], ps3[:, :n], AF.Relu,
                                bias=b3[:, oc:oc + 1])
                        else:
                            nc.vector.tensor_scalar(
                                m3[:, oc, n0:n0 + n], ps3[:, :n],
                                b3[:, oc:oc + 1], 0.0,
                                op0=ALU.add, op1=ALU.max)
                    elif c % 2 == 0:
                        nc.vector.tensor_copy(m3[:, oc, n0:n0 + n],
                                              ps3[:, :n])
                    else:
                        nc.scalar.activation(m3[:, oc, n0:n0 + n],
                                             ps3[:, :n], AF.Identity)
                # stagger the two oc pool bursts one chunk apart so the
                # vector queue never backs up enough to stall PSUM rotation
                if c in (4, 8, 12, 16):
                    pool_group((c - 4) // 4, (0,))
                if c in (5, 9, 13):
                    pool_group((c - 5) // 4, (1,))
            pool_group(3, (1,))

            p_m23.release()
            p_m1.release()

            # GRU weights/buffers pool (small, lives to the end)
            p_gru = tc.alloc_tile_pool(name="gru", bufs=1)
            wihT = p_gru.tile([128, 2, 3, 4, 128], bf16)
            nc.sync.dma_start(wihT[:], wihT_d[:])
            brz = p_gru.tile([128, 2, 2], f32)
            nc.sync.dma_start(brz[:], brz_d[:])
            bng = p_gru.tile([128, 2], f32)
            nc.sync.dma_start(bng[:], bng_d[:])
            whhT = p_gru.tile([128, 2, 3, 128], bf16)
            nc.sync.dma_start(whhT[:], whhT_d[:])
            E2 = p_gru.tile([128, 32, 2], bf16)
            nc.sync.dma_start(E2[:], E2_d[:])
            I128 = p_gru.tile([128, 128], bf16)
            nc.sync.dma_start(I128[:], I128_d[:])
            bhr = p_gru.tile([1, 2, 128], bf16)
            nc.sync.dma_start(bhr[:], bhr_d[:])
            clsT = p_gru.tile([128, 2, 5], bf16)
            nc.sync.dma_start(clsT[:], clsT_d[:])
            clsb5 = p_gru.tile([5, 1], f32)
            nc.sync.dma_start(clsb5[:], clsb5_d[:])
            one11 = p_gru.tile([1, 1], bf16)
            nc.vector.memset(one11[:], 1.0)
            ggx_pre = p_gru.tile([128, 2, 2, 32], bf16)  # (d, g, t) biases
            gbT = p_gru.tile([128, 128], bf16)           # transposed biases
            ggxn = p_gru.tile([128, 2, 32], f32)
            hall = p_gru.tile([128, 2, 33], bf16)
            nc.vector.memset(hall[:], 0.0)
            hsum = p_gru.tile([128, 2, 2], bf16)
            nc.vector.memset(hsum[:], 0.0)
            pgxs = p_gru.tile([128, 6, 32], f32)
            ftr2 = p_gru.tile([128, 4, 32], bf16)   # reduced features

            # ============== FC FUSION CONV (fp8 DoubleRow) ================

            def fc_oc(oc):
                for rg in range(4):
                    ps = psum_mm([128, 8, 64])
                    first = True
                    for dy in range(3):
                        for dx in range(3):
                            nc.tensor.matmul(
                                ps[:],
                                fcm[:, :, 3 * dy + dx, oc, :],
                                xmel[:, :, rg * 8 + dy:rg * 8 + dy + 8,
                                     dx:dx + 64],
                                start=first, stop=False, perf_mode=DR)
                            first = False
                    # wave half + fb via CT rows (broadcast across rows)
                    nc.tensor.matmul(
                        ps[:], CT[:, 0, 128 * oc:128 * (oc + 1)], dlt[:],
                        start=False, stop=not (rg == 0 or rg == 3),
                        skip_group_check=True)
                    if rg == 0:
                        nc.tensor.matmul(
                            ps[:, 0:1, :],
                            CT[:, 1, 128 * oc:128 * (oc + 1)],
                            dlt[:, 0:1, :], start=False, stop=True,
                            skip_group_check=True)
                    if rg == 3:
                        nc.tensor.matmul(
                            ps[:, 7:8, :],
                            CT[:, 2, 128 * oc:128 * (oc + 1)],
                            dlt[:, 0:1, :], start=False, stop=True,
                            skip_group_check=True)
                    xft = sc.tile([128, 8, 64], bf16, tag="xf", bufs=3)
                    nc.scalar.activation(xft[:], ps[:], AF.Relu)
                    nc.vector.tensor_reduce(
                        featp[:, oc, rg * 8:rg * 8 + 8], xft[:],
                        axis=AX.X, op=ALU.add)
                nc.vector.tensor_copy(ftr[:, oc, :], featp[:, oc, :])

            ccin = [dram.tile([128, 2, 32], bf16, tag=f"ci{i}",
                              name=f"ccin{i}") for i in range(2)]
            ccout = [dram.tile([128, 2, 32], bf16, tag=f"co{i}",
                               name=f"ccout{i}") for i in range(2)]

            def cc_half(i):
                # AllReduce one bf16 feat half (16KB); pipelined on the CC
                # engine so only the second one's latency is exposed
                nc.sync.dma_start(ccin[i][:], ftr[:, 2 * i:2 * i + 2, :])
                nc.gpsimd.collective_compute(
                    "AllReduce", ALU.add,
                    replica_groups=[list(range(NCORES))],
                    ins=[ccin[i].opt()], outs=[ccout[i].opt()])
                nc.sync.dma_start(ftr2[:, 2 * i:2 * i + 2, :],
                                  ccout[i][:])

            def gx_chunk(kk):
                psk = psp.tile([128, 6, 32], f32, tag="pgx", bufs=1,
                               name="psk")
                for d in range(2):
                    for g in range(3):
                        nc.tensor.matmul(psk[:, 3 * d + g, :],
                                         wihT[:, d, g, kk, :],
                                         ftr2[:, kk, :],
                                         start=True, stop=True)
                if kk == 0:
                    nc.vector.tensor_copy(pgxs[:], psk[:])
                else:
                    nc.vector.tensor_tensor(pgxs[:], pgxs[:], psk[:],
                                            op=ALU.add)

            fc_oc(0)
            fc_oc(1)
            cc_half(0)
            fc_oc(2)
            fc_oc(3)
            cc_half(1)
            gx_chunk(0)
            gx_chunk(1)
            # keep the tensor engine warm through the CC tail: idle
            # windows trigger a 50% duty-cycle downclock taxing the GRU 2x
            for dk in range(60):
                psd = psum_mm([128, 8, 64])
                nc.tensor.matmul(psd[:], fcm[:, :, dk % 9, 0, :],
                                 xmel[:, :, 8 + (dk % 3):16 + (dk % 3),
                                      1:65],
                                 start=True, stop=True, perf_mode=DR)
            gx_chunk(2)
            gx_chunk(3)

            # gate biases: r rows = pgx_r + brz_r; z rows = -(pgx_z) + brz_z
            # (brz z-col is pre-negated host-side); built per (d,g) then
            # DMA-transposed so the K=64 bias matmul can read them as rows
            nc.scalar.activation(ggx_pre[:, 0, 0, :], pgxs[:, 0, :],
                                 AF.Identity, bias=brz[:, 0, 0:1])
            nc.scalar.activation(ggx_pre[:, 0, 1, :], pgxs[:, 1, :],
                                 AF.Identity, bias=brz[:, 0, 1:2],
                                 scale=-1.0)
            nc.scalar.activation(ggxn[:, 0, :], pgxs[:, 2, :],
                                 AF.Identity, bias=bng[:, 0:1])
            nc.vector.tensor_scalar(ggx_pre[:, 1, 0, :], pgxs[:, 3, :],
                                    brz[:, 1, 0:1], 0.0,
                                    op0=ALU.add, op1=ALU.bypass)
            nc.vector.tensor_scalar(ggx_pre[:, 1, 1, :], pgxs[:, 4, :],
                                    brz[:, 1, 1:2], -1.0,
                                    op0=ALU.subtract, op1=ALU.mult)
            nc.vector.tensor_scalar(ggxn[:, 1, :], pgxs[:, 5, :],
                                    bng[:, 1:2], 0.0,
                                    op0=ALU.add, op1=ALU.bypass)
            # transpose the (d,g,t)-bias block onto partitions via the PE
            psT = psp.tile([128, 128], bf16, tag="aux", bufs=2,
                           name="psT")
            nc.tensor.transpose(psT[:],
                                ggx_pre[:].rearrange("p a b c -> p (a b c)"),
                                I128[:])
            nc.vector.tensor_copy(gbT[:], psT[:])

            if DEBUG_TAPS:
                nc.sync.dma_start(dbg_xw1_d[:], xw1[:])
                nc.sync.dma_start(dbg_CT_d[:], CT[:])
                nc.sync.dma_start(dbg_featp_d[:], featp[:])
                nc.sync.dma_start(dbg_pgx_d[:], pgxs[:])
                nc.sync.dma_start(dbg_gbT_d[:], gbT[:])

            # ============== GRU (replicated) =============================
            if DEBUG_TAPS:
                dbg_ps0 = p_gru.tile([128, 3, 2], f32, name="dbgps0")
            for s in range(32):
                for d in range(2):
                    t = s if d == 0 else 31 - s
                    # mm tag is idle by now; its 3-deep rotation gives the
                    # GRU more PSUM lookahead than the 2-deep aux
                    ps = psum_mm([128, 3])
                    # bias matmul: gbT rows (d,g,t) selected by E2 -> r,z
                    nc.tensor.matmul(ps[:, 0:2],
                                     gbT[64 * d:64 * d + 64, :],
                                     E2[64 * d:64 * d + 64, t, :],
                                     start=True, stop=False,
                                     skip_group_check=True)
                    # n-gate psum constant: bhh_n row. start=False: the
                    # bias MM's start already marked this zero-region
                    # pending, so this write still zero-overwrites; a
                    # second start=True would re-arm pending-zero over the
                    # bias cols and drop them.
                    nc.tensor.matmul(ps[:, 2:3], bhr[0:1, d, :],
                                     one11[:], start=False, stop=False,
                                     skip_group_check=True)
                    for g in (0, 1, 2):  # r,z first (unblocks sigmoid)
                        nc.tensor.matmul(ps[:, g:g + 1], whhT[:, d, g, :],
                                         hall[:, d, s:s + 1],
                                         start=False, stop=True,
                                         skip_group_check=True)
                    if DEBUG_TAPS and s == 0:
                        nc.vector.tensor_copy(dbg_ps0[:, :, d], ps[:])
                    rz = sc.tile([128, 2], f32, tag="rz", bufs=6)
                    nc.scalar.activation(rz[:], ps[:, 0:2], AF.Sigmoid)
                    # n = tanh(r*(gh_n + bhh_n) + gx_n): the r-multiply
                    # rides the ACT's scale operand, no vector hop
                    nt = sc.tile([128, 1], f32, tag="nt", bufs=6)
                    nc.scalar.activation(nt[:], ps[:, 2:3], AF.Tanh,
                                         bias=ggxn[:, d, t:t + 1],
                                         scale=rz[:, 0:1])
                    # rz[:,1] holds z' = 1-z.  hmn = h*z' - h (off critical
                    # path); h' = n*z' - hmn = (1-z)*n + z*h
                    hmn = sc.tile([128, 1], f32, tag="hmn", bufs=6)
                    nc.vector.scalar_tensor_tensor(
                        hmn[:], hall[:, d, s:s + 1], rz[:, 1:2],
                        hall[:, d, s:s + 1], op0=ALU.mult,
                        op1=ALU.subtract)
                    nc.vector.scalar_tensor_tensor(
                        hall[:, d, s + 1:s + 2], nt[:], rz[:, 1:2],
                        hmn[:], op0=ALU.mult, op1=ALU.subtract)

            if DEBUG_TAPS:
                nc.sync.dma_start(dbg_hall_d[:], hall[:])
                nc.sync.dma_start(dbg_E2_d[:], E2[:])
                nc.sync.dma_start(dbg_ps0_d[:], dbg_ps0[:])
            with nc.allow_low_precision(reason="hsum mean of 32 states"):
                nc.vector.tensor_reduce(hsum[:, :, 0], hall[:, :, 1:33],
                                        axis=AX.X, op=ALU.add)
            psc = psum_aux([5, 2])
            for d in range(2):
                nc.tensor.matmul(psc[:], clsT[:, d, :], hsum[:, d, :],
                                 start=(d == 0), stop=(d == 1))
            lgt = sc.tile([5, 1], f32, tag="lgt")
            nc.scalar.activation(lgt[:], psc[:, 0:1], AF.Identity,
                                 bias=clsb5[:, 0:1])
            nc.sync.dma_start(out_d[0:1, :].rearrange("a p -> p a"), lgt[:])
            p_gru.release()

    nc.compile()
    return nc


def _prep_inputs(inputs):
    """Build the 8 per-core input maps from the full model inputs."""
    import ml_dtypes
    f = np.float32
    bf = ml_dtypes.bfloat16
    f8 = ml_dtypes.float8_e4m3

    def b(x):
        return np.ascontiguousarray(np.asarray(x, f)).astype(bf)

    def q8(x):
        return np.ascontiguousarray(
            np.clip(np.asarray(x, f), -240.0, 240.0)).astype(f8)

    wave = np.asarray(inputs["waveform"], f).reshape(16000)
    logmel = np.asarray(inputs["logmel"], f).reshape(64, 1024)

    wp = np.zeros(16015, f)
    wp[3:16003] = wave
    wP = b(wp.reshape(3203, 5).T)                      # [5, 3203]

    R = _resize_matrix(50, 512)
    Rp = np.zeros((50, 514), f)
    Rp[:, 1:513] = R

    lmp = np.pad(logmel, ((1, 1), (4, 6)))

    w1m = b(np.asarray(inputs["mc1"], f).reshape(64, 9).T)
    s1 = np.asarray(inputs["bn1g"], f) / np.sqrt(
        np.asarray(inputs["bn1v"], f) + 1e-5)
    b1 = (np.asarray(inputs["mb1"], f) - np.asarray(inputs["bn1m"], f)) * s1 \
        + np.asarray(inputs["bn1b"], f)
    mc2 = np.asarray(inputs["mc2"], f)              # [128, 64, 3, 3]
    # w2m [128, 3(dx), 2(dy-pair), 128]: pair 0 = dy rows (0,1) packed on
    # partitions, pair 1 = dy row 2 on partitions 0-63 (64-127 zero)
    w2m = np.zeros((128, 3, 2, 128), f)
    for dx in range(3):
        w2m[0:64, dx, 0, :] = mc2[:, :, 0, dx].T
        w2m[64:128, dx, 0, :] = mc2[:, :, 1, dx].T
        w2m[0:64, dx, 1, :] = mc2[:, :, 2, dx].T
    s2 = np.asarray(inputs["bn2g"], f) / np.sqrt(
        np.asarray(inputs["bn2v"], f) + 1e-5)
    b2 = (np.asarray(inputs["mb2"], f) - np.asarray(inputs["bn2m"], f)) * s2 \
        + np.asarray(inputs["bn2b"], f)
    w3m = b(np.asarray(inputs["mc3"], f).reshape(256, 128).T
            .reshape(128, 2, 128))
    b3 = np.ascontiguousarray(
        np.asarray(inputs["mb3"], f).reshape(2, 128).T)

    fc = np.asarray(inputs["fc"], f)                   # [512,512,3,3]
    fcmel = fc[:, 256:, :, :]
    fcm = q8(fcmel.reshape(4, 128, 2, 128, 9).transpose(3, 2, 4, 0, 1))
    fcwave = fc[:, :256, :, :]                         # [512, 256, 3, 3]
    # fcwV [128(chp), 2(ch), 3(dx), 3(var), 512(oc)]
    fw = fcwave.reshape(512, 2, 128, 3, 3)             # oc, ch, chp, dy, dx
    fcwV = np.zeros((128, 2, 3, 3, 512), f)
    fcwV[:, :, :, 0, :] = fw.sum(axis=3).transpose(2, 1, 3, 0)
    fcwV[:, :, :, 1, :] = -fw[:, :, :, 0, :].transpose(2, 1, 3, 0)
    fcwV[:, :, :, 2, :] = -fw[:, :, :, 2, :].transpose(2, 1, 3, 0)
    fcwV = b(fcwV)
    fbrow = b(np.asarray(inputs["fb"], f).reshape(1, 512))
    dlt = b(np.broadcast_to(np.eye(64, dtype=f)[:, None, :], (64, 8, 64)))

    wc1 = np.asarray(inputs["wc1"], f).reshape(64, 11)
    w1w = np.zeros((5, 3, 64), f)
    for tap in range(11):
        w1w[tap % 5, tap // 5, :] = wc1[:, tap]
    w1w = b(w1w)
    w2w = b(np.asarray(inputs["wc2"], f).reshape(128, 64, 5)
            .transpose(1, 2, 0))
    w3w = b(np.asarray(inputs["wc3"], f).reshape(256, 128, 5)
            .transpose(1, 2, 0).reshape(128, 5, 2, 128))
    wb3w = np.ascontiguousarray(
        np.asarray(inputs["wb3"], f).reshape(2, 128).T)
    w4wT = b(np.asarray(inputs["wc4"], f).reshape(256, 256, 5)
             .transpose(1, 2, 0).reshape(2, 128, 5, 256)
             .transpose(1, 0, 2, 3))
    wb4row = b(np.asarray(inputs["wb4"], f).reshape(1, 256))
    one50 = b(np.ones((1, 50), f))

    def gru_prep(d):
        wih = np.asarray(inputs[f"wih_{d}"], f) / 512.0
        whh = np.asarray(inputs[f"whh_{d}"], f)
        bih = np.asarray(inputs[f"bih_{d}"], f)
        bhh = np.asarray(inputs[f"bhh_{d}"], f)
        wihT = np.ascontiguousarray(
            wih.reshape(3, 128, 4, 128).transpose(3, 0, 2, 1))
        whhT = np.ascontiguousarray(
            whh.reshape(3, 128, 128).transpose(2, 0, 1))
        brz = (bih + bhh)[:256].reshape(2, 128).T
        return wihT, whhT, brz, bih[256:], bhh[256:]

    wihT_f, whhT_f, brz_f, bn_f, bhn_f = gru_prep("f")
    wihT_b, whhT_b, brz_b, bn_b, bhn_b = gru_prep("b")
    wihT = b(np.stack([wihT_f, wihT_b], axis=1))
    whhT_np = np.stack([whhT_f, whhT_b], axis=1)
    whhT_np[:, :, 1, :] *= -1.0  # z gate: sigma(-(a)) = 1-z via plain ACT
    whhT = b(whhT_np)
    brz = np.ascontiguousarray(np.stack([brz_f, brz_b], axis=1))
    brz[:, :, 1] *= -1.0
    bng = np.ascontiguousarray(np.stack([bn_f, bn_b], axis=1))
    clsW = np.asarray(inputs["clsW"], f) / 32.0
    clsT = b(clsW.reshape(5, 2, 128).transpose(2, 1, 0))
    clsb5 = np.asarray(inputs["clsb"], f).reshape(5, 1)

    bhr = b(np.stack([bhn_f, bhn_b], axis=0).reshape(1, 2, 128))

    # E2 selector [128, 32, 2]: row p = 64d + 32g + tt picks (g, tt)
    E2 = np.zeros((128, 32, 2), f)
    for d in range(2):
        for g in range(2):
            for tt in range(32):
                E2[64 * d + 32 * g + tt, tt, g] = 1.0
    E2 = b(E2)

    shared = dict(
        wP=wP, w1m=w1m, bn1s=s1.reshape(64, 1), bn1b=b1.reshape(64, 1),
        bhr=bhr,
        w2m=q8(w2m), bn2s=s2.reshape(128, 1), bn2b=b2.reshape(128, 1),
        w3m=w3m, b3=b3, fcm=fcm, fcwV=fcwV, fbrow=fbrow, dlt=dlt,
        w1w=w1w, wb1=np.asarray(inputs["wb1"], f).reshape(64, 1),
        w2w=w2w, wb2=np.asarray(inputs["wb2"], f).reshape(128, 1),
        w3w=w3w, wb3w=wb3w, w4wT=w4wT, wb4row=wb4row, one50=one50,
        wihT=wihT, brz=brz, bng=bng, whhT=whhT, E2=E2,
        I128=b(np.eye(128, dtype=f)),
        clsT=clsT, clsb5=clsb5, zed=np.zeros((128, 2), f),
    )
    in_maps = []
    for i in range(NCORES):
        m = dict(shared)
        lms = lmp[:, 128 * i:128 * i + 138]
        lmI = np.empty((9, MW), f)
        for dy in range(3):
            for dx in range(3):
                lmI[3 * dy + dx] = lms[dy:dy + 64, dx:dx + 136].reshape(-1)
        m["lmI"] = b(lmI)
        m["Ri"] = b(Rp[:, 64 * i:64 * i + 66])
        mk = np.ones((128, 2), f)
        if i == 0:
            mk[:, 0] = 0.0
        if i == NCORES - 1:
            mk[:, 1] = 0.0
        m["msk"] = mk
        in_maps.append(m)
    return in_maps


def kernel(**inputs):
    global LAST_RESULTS
    _ensure_concourse()
    from concourse import bass_utils

    if "nc" not in _CACHE:
        _CACHE["nc"] = _build()
    nc = _CACHE["nc"]
    in_maps = _prep_inputs(inputs)
    res = bass_utils.run_bass_kernel_spmd(
        nc, in_maps, core_ids=list(range(NCORES)))
    LAST_RESULTS = res
    return res.results[0]["out"]


if __name__ == "__main__":
    _ensure_concourse()
    _build()
    print("build + compile OK")
